# revision 17
# baseline (speedup 1.0000x reference)
"""Banded soft-DTW loss (normalize=True) Trainium2 Bass kernel, v2.

Problem: x, y [32, 512, 4] f32 -> loss [32] f32
  loss = softdtw(x,y) - 0.5*(softdtw(x,x) + softdtw(y,y)), gamma=2, band=50.

Strategy v2 — forward/backward split (halves the serial DP chain):
  * The soft-DTW path-sum factors at the middle cut:
      S_total = sum_j F[255,j] * (B[256,j] + B[256,j+1])
    where F is the forward exp-space DP after rows 0..255 and B the
    backward DP, which equals the forward DP of the REVERSED sequences.
  * Cores 0-3 run forward halves, cores 4-7 run the same program on
    host-reversed inputs. Each core: 24 DP problems (8 batches x
    {xy,xx,yy}) x 256 rows, batched across 24 SBUF partitions.
  * Cost matrices: one K=6 augmented matmul per (problem, 128-row chunk)
    computes G' = -D/2 over a 228-wide j-window; ACT exp -> E = exp(-D/2)
    bf16; DRAM round-trip extracts the 101-wide diagonal band per row.
  * Exp-space DP row scan: S_i[t] = E_i[t]*(S_i[t-1] + S_{i-1}[t+1] +
    S_{i-1}[t]) via tensor_add + tensor_tensor_scan per row on DVE.
    Rescale by the diagonal cell every RS rows (log accumulated).
  * Each core outputs its boundary row + log-rescale sum; the host does
    the tiny O(B*W) combine.
"""
import os
import sys
from contextlib import ExitStack

import numpy as np

for _p in ("/opt/trn_rl_repo", "/root/.axon_site/_ro/trn_rl_repo"):
    if os.path.isdir(_p) and _p not in sys.path:
        sys.path.append(_p)

import concourse.bass as bass
import concourse.bacc as bacc
import concourse.mybir as mybir
import concourse.tile as tile
from concourse.bass_utils import run_bass_kernel_spmd

F32 = mybir.dt.float32
BF16 = mybir.dt.bfloat16
ALU = mybir.AluOpType
ACTF = mybir.ActivationFunctionType

N = 512            # full sequence length
HN = 256           # rows per half (per core)
DIM = 4
NBAT = 8           # batch elements per core
NSEQ = 16          # sequences per core (8 x + 8 y)
NPROB = 24         # DP problems per core (xy, xx, yy for 8 batches)
NCORE = 8
BW = 50            # band half width
W = 101            # band window width per row
SW = 104           # S tile width (101 + 3 zero guard cols)
NCHUNK = 2         # 128-row chunks per half
WIN = 228          # matmul j-window: 128 + 101 - 1
ACOLS = HN         # a-side cols per sequence
AV = HN + BW       # valid b prefix length (306)
BCOLS = 356        # b cols per sequence: 50 left pad + 306
RS = 8             # rescale cadence (rows)
GROWS = 32         # E-tile group rows
NGRP = HN // GROWS
NEVT = HN // RS    # 32 rescale events
CAP = 1e30
NEG = -20000.0     # pad dot-product value -> exp() == 0
HALF = NPROB // 2  # problems per interleaved DP stream


def _groups():
    # (chunk, first_row, nrows): 8-row leading groups so the DP can start
    # as soon as the first few rows of E are gathered.
    gs = []
    for c in range(NCHUNK):
        sizes = [8, 8, 8, 8, 32, 32, 32] if c == 0 else [32, 32, 32, 32]
        r = c * 128
        for n in sizes:
            gs.append((c, r, n))
            r += n
    return gs


GRPS = _groups()
ROW2G = {}
for _gi, (_c, _r0, _n) in enumerate(GRPS):
    for _r in range(_r0, _r0 + _n):
        ROW2G[_r] = (_gi, _r - _r0)


def _build_nc():
    nc = bacc.Bacc("TRN2", target_bir_lowering=False, debug=False)
    xt = nc.dram_tensor("xt", [NBAT, DIM, N], F32, kind="ExternalInput").ap()
    yt = nc.dram_tensor("yt", [NBAT, DIM, N], F32, kind="ExternalInput").ap()
    nhc = nc.dram_tensor("nhc", [5, 12], F32, kind="ExternalInput").ap()
    out = nc.dram_tensor("out", [NPROB, W + 1], F32, kind="ExternalOutput").ap()

    with tile.TileContext(nc) as tc, ExitStack() as ctx:
        _emit(ctx, tc, xt, yt, nhc, out)
    nc.compile()
    return nc


def _emit(ctx, tc, xt, yt, nhc, out):
    nc = tc.nc

    const = ctx.enter_context(tc.tile_pool(name="const", bufs=1))
    winp = ctx.enter_context(tc.tile_pool(name="winp", bufs=26))
    ps_win = ctx.enter_context(tc.tile_pool(name="ps_win", bufs=6, space="PSUM"))
    ps_misc = ctx.enter_context(tc.tile_pool(name="ps_misc", bufs=1, space="PSUM"))
    dramp = ctx.enter_context(tc.tile_pool(name="dramp", bufs=1, space="DRAM"))
    epool = ctx.enter_context(tc.tile_pool(name="epool", bufs=1))

    scratch = dramp.tile([NPROB, NCHUNK, 128, WIN], BF16)
    e_tiles = [
        (epool.tile([HALF, n * W], BF16, name=f"eA{gi}", tag=f"eA{gi}"),
         epool.tile([HALF, n * W], BF16, name=f"eB{gi}", tag=f"eB{gi}"))
        for gi, (c, r0, n) in enumerate(GRPS)
    ]

    QQ = [nc.sync, nc.scalar, nc.gpsimd]

    # DP state tiles are allocated BEFORE the prep pool so they do not
    # reuse its SBUF (which would add a WAR dependency on the whole prep).
    spool = ctx.enter_context(tc.tile_pool(name="spool", bufs=1))
    cbp = ctx.enter_context(tc.tile_pool(name="cbp", bufs=2))
    mA = const.tile([HALF, NEVT], F32)
    mB = const.tile([HALF, NEVT], F32)
    onesd = const.tile([HALF, 1], F32)
    nc.vector.memset(onesd[:], 1.0)
    sA = [spool.tile([HALF, SW], BF16, tag=f"sA{k}", name=f"sA{k}")
          for k in range(3)]
    sB = [spool.tile([HALF, SW], BF16, tag=f"sB{k}", name=f"sB{k}")
          for k in range(3)]
    for s in sA + sB:
        nc.vector.memset(s[:], 0.0)

    # ---- Phase 0+1 fused: K=6 operand prep pipelined with chunk-0 ---------
    # a6 rows: [a0..a3, ones, -|a|^2/2]; b6 rows: [b0..b3, -|b|^2/2 (NEG
    # pads), ones].  G' = a6 . b6 = a.b - |a|^2/2 - |b|^2/2 = -D/2.
    # Norm rows are produced as 6-partition PSUM matmul outputs (rows 0-3
    # zero) and ACT-copied over a6/b6 BEFORE the feature casts land.
    with tc.tile_pool(name="pre", bufs=1) as pre:
        a6 = pre.tile([6, NSEQ * ACOLS], BF16)
        b6 = pre.tile([6, NSEQ * BCOLS], BF16)
        nh_f = pre.tile([5, 12], F32)
        nh_bf = pre.tile([5, 12], BF16)
        stag = pre.tile([4, NSEQ * AV], F32)
        sq = pre.tile([5, NSEQ * AV], BF16)
        ones_stg = pre.tile([1, NSEQ * AV], BF16)
        npad = pre.tile([1, NBAT * BW * 2], BF16)

        # nha col4 picks the ones row, col5 = -0.5*sum(sq); nhb swapped.
        # Pattern comes in as a tiny host input (engine ops cannot write
        # single partitions above 0).
        nc.gpsimd.dma_start(nh_f[:], nhc)
        nc.vector.tensor_copy(nh_bf[:], nh_f[:])
        nha = nh_bf[:, 0:6]
        nhb = nh_bf[:, 6:12]
        nc.vector.memset(ones_stg[:], 1.0)
        nc.vector.memset(npad[:], NEG)
        # sq row 4 = ones (via DMA: engine can't write partition 4 alone)
        nc.gpsimd.dma_start(sq[4:5, :], ones_stg[:])
        # b6 row-4 NEG pads and row-5 ones pads for all seqs in 2 DMAs
        b45 = b6[4:6, :].rearrange("p (s c) -> p s c", c=BCOLS)
        nc.gpsimd.dma_start(
            b45[0:1, :, 0:BW],
            npad[:].rearrange("p (s c) -> p s c", c=BW))
        nc.gpsimd.dma_start(
            b45[1:2, :, 0:BW],
            ones_stg[:, 0:NSEQ * BW].rearrange("p (s c) -> p s c", c=BW))

        st3 = stag[:].rearrange("p (s c) -> p s c", c=AV)
        sq3 = sq[0:4, :].rearrange("p (s c) -> p s c", c=AV)
        a3 = a6[0:4, :].rearrange("p (s c) -> p s c", c=ACOLS)
        b3 = b6[0:4, :].rearrange("p (s c) -> p s c", c=BCOLS)
        xsrc = xt.transpose([1, 0, 2])
        ysrc = yt.transpose([1, 0, 2])
        nc.sync.dma_start(st3[:, 0:NBAT, :], xsrc[:, :, 0:AV])
        nc.scalar.dma_start(st3[:, NBAT:NSEQ, :], ysrc[:, :, 0:AV])

        pairs = [(b, NBAT + b) for b in range(NBAT)] \
            + [(b, b) for b in range(NBAT)] \
            + [(NBAT + b, NBAT + b) for b in range(NBAT)]
        sc_handle = scratch[:].tensor

        ew_tiles = {}

        def emit_problem(pi, c):
            sa, sb = pairs[pi]
            pw = ps_win.tile([128, WIN], F32, name=f"pw{pi}_{c}", tag="pw")
            nc.tensor.matmul(
                pw[:],
                a6[:, sa * ACOLS + c * 128:sa * ACOLS + (c + 1) * 128],
                b6[:, sb * BCOLS + c * 128:sb * BCOLS + c * 128 + WIN],
                start=True, stop=True,
            )
            ew = winp.tile([128, WIN], BF16, name=f"ew{pi}_{c}", tag="ew")
            nc.scalar.activation(ew[:], pw[:], ACTF.Exp)
            ew_tiles[pi] = ew

        # per-seq prep pipeline: mul (DVE) -> 2 norm matmuls (PE) ->
        # 2 ACT copies -> feature casts (DVE); then this batch's chunk-0
        # problems immediately so the E pipeline starts while later
        # sequences are still being prepped.
        for b in range(NBAT):
            for s in (b, NBAT + b):
                nc.vector.tensor_mul(sq3[:, s, :], st3[:, s, :], st3[:, s, :])
                pna = ps_misc.tile([6, ACOLS], F32, name=f"pna{s}", tag="pna")
                pnb = ps_misc.tile([6, AV], F32, name=f"pnb{s}", tag="pnb")
                nc.tensor.matmul(
                    pna[:], nha[:], sq[:, s * AV:s * AV + ACOLS],
                    start=True, stop=True)
                nc.tensor.matmul(
                    pnb[:], nhb[:], sq[:, s * AV:(s + 1) * AV],
                    start=True, stop=True)
                nc.scalar.copy(a6[:, s * ACOLS:(s + 1) * ACOLS], pna[:])
                nc.scalar.copy(b6[:, s * BCOLS + BW:(s + 1) * BCOLS], pnb[:])
                nc.vector.tensor_copy(a3[:, s, :], st3[:, s, 0:ACOLS])
                nc.vector.tensor_copy(b3[:, s, BW:BCOLS], st3[:, s, :])
                nc.vector.memset(b3[:, s, 0:BW], 0.0)
            emit_problem(b, 0)            # xy
            emit_problem(NBAT + b, 0)     # xx
            emit_problem(2 * NBAT + b, 0) # yy

        def emit_slab_writes_gathers(c):
            # slab-major: each group's rows written for every problem, then
            # that group's gather — the DP's first group unblocks right
            # after the exps instead of after all full-window writes.
            qi = 0
            for gi, (cc, r0, n) in enumerate(GRPS):
                if cc != c:
                    continue
                rl = r0 - c * 128
                for pi in range(NPROB):
                    QQ[pi % 3].dma_start(
                        scratch[pi, c, rl:rl + n],
                        ew_tiles[pi][rl:rl + n, :])
                for half in range(2):
                    e3 = e_tiles[gi][half][:].rearrange(
                        "p (r t) -> p r t", t=W)
                    for sub in range(2):
                        p0 = half * HALF + sub * 6
                        src = bass.AP(
                            sc_handle,
                            p0 * (NCHUNK * 128 * WIN) + c * 128 * WIN
                            + rl * (WIN + 1),
                            [[NCHUNK * 128 * WIN, 6], [WIN + 1, n], [1, W]],
                        )
                        QQ[qi % 3].dma_start(e3[sub * 6:sub * 6 + 6], src)
                        qi += 1

        emit_slab_writes_gathers(0)
        for pi in range(NPROB):
            emit_problem(pi, 1)
        emit_slab_writes_gathers(1)

    # ---- Phase 3: exp-space row-scan DP, two interleaved streams ----------
    # A = problems 0-11, B = 12-23 in separate partition-0 tiles; the B ops
    # fill the DVE pipeline while A's completion semaphores propagate.
    for i in range(HN):
        gi, rl = ROW2G[i]
        eA = e_tiles[gi][0][:].rearrange("p (r t) -> p r t", t=W)[:, rl, :]
        eB = e_tiles[gi][1][:].rearrange("p (r t) -> p r t", t=W)[:, rl, :]
        ctA = cbp.tile([HALF, W], BF16, tag="cA", name=f"cA{i}")
        ctB = cbp.tile([HALF, W], BF16, tag="cB", name=f"cB{i}")
        if i == 0:
            nc.vector.memset(ctA[:], 0.0)
            nc.vector.memset(ctB[:], 0.0)
            nc.vector.memset(ctA[:, BW:BW + 1], 1.0)
            nc.vector.memset(ctB[:, BW:BW + 1], 1.0)
        else:
            spA, spB = sA[(i - 1) % 3], sB[(i - 1) % 3]
            nc.vector.tensor_add(ctA[:], spA[:, 1:W + 1], spA[:, 0:W])
            nc.vector.tensor_add(ctB[:], spB[:, 1:W + 1], spB[:, 0:W])
        stA, stB = sA[i % 3], sB[i % 3]
        # state = (c[t] + state) * E[t]  — the full soft-DTW row recurrence
        nc.vector.tensor_tensor_scan(
            stA[:, 0:W], ctA[:], eA, 0.0, ALU.add, ALU.mult)
        nc.vector.tensor_tensor_scan(
            stB[:, 0:W], ctB[:], eB, 0.0, ALU.add, ALU.mult)
        if i % RS == RS - 1:
            ev = i // RS
            nc.vector.reciprocal(mA[:, ev:ev + 1], stA[:, BW:BW + 1])
            nc.vector.reciprocal(mB[:, ev:ev + 1], stB[:, BW:BW + 1])
            nc.vector.tensor_scalar(
                stA[:, 0:W], stA[:, 0:W], mA[:, ev:ev + 1], CAP,
                ALU.mult, ALU.min)
            nc.vector.tensor_scalar(
                stB[:, 0:W], stB[:, 0:W], mB[:, ev:ev + 1], CAP,
                ALU.mult, ALU.min)

    # ---- Phase 4: readout — boundary row + log-rescale sum ----------------
    lnA = const.tile([HALF, NEVT], F32)
    lnB = const.tile([HALF, NEVT], F32)
    obufA = const.tile([HALF, W + 1], F32)
    obufB = const.tile([HALF, W + 1], F32)
    nc.scalar.activation(lnA[:], mA[:], ACTF.Ln)
    nc.scalar.activation(lnB[:], mB[:], ACTF.Ln)
    nc.vector.reduce_sum(obufA[:, W:W + 1], lnA[:], axis=mybir.AxisListType.X)
    nc.vector.reduce_sum(obufB[:, W:W + 1], lnB[:], axis=mybir.AxisListType.X)
    nc.scalar.copy(obufA[:, 0:W], sA[(HN - 1) % 3][:, 0:W])
    nc.scalar.copy(obufB[:, 0:W], sB[(HN - 1) % 3][:, 0:W])
    nc.sync.dma_start(out[0:HALF, :], obufA[:])
    nc.scalar.dma_start(out[HALF:NPROB, :], obufB[:])


_NC_CACHE = None


def _get_nc():
    global _NC_CACHE
    if _NC_CACHE is None:
        _NC_CACHE = _build_nc()
    return _NC_CACHE


def _nhc_np():
    # [5, 12] = nha | nhb, contraction rows [sq0..sq3, ones]
    v = np.zeros((5, 12), np.float32)
    v[0:4, 5] = -0.5   # nha col5: -|a|^2/2
    v[4, 4] = 1.0      # nha col4: ones row
    v[0:4, 6 + 4] = -0.5  # nhb col4: -|b|^2/2
    v[4, 6 + 5] = 1.0     # nhb col5: ones row
    return v


def _in_maps(x, y):
    """Per-core inputs: cores 0-3 forward batches 8g..8g+7, cores 4-7 the
    same batches with sequences reversed (backward half)."""
    maps = []
    for c in range(NCORE):
        g = c % 4
        xs = x[NBAT * g:NBAT * (g + 1)].transpose(0, 2, 1)
        ys = y[NBAT * g:NBAT * (g + 1)].transpose(0, 2, 1)
        if c >= 4:
            xs = xs[:, :, ::-1]
            ys = ys[:, :, ::-1]
        maps.append({
            "xt": np.ascontiguousarray(xs),
            "yt": np.ascontiguousarray(ys),
            "nhc": _nhc_np(),
        })
    return maps


def _combine(outs):
    """Host combine: S_total = sum_t F[t]*(Bp[101-t] + Bp[100-t]);
    loss = R_xy - (R_xx + R_yy)/2 with R = -2*(ln S - tsumF - tsumB)."""
    loss = np.zeros(NBAT * 4, np.float32)
    for g in range(4):
        Fo = np.asarray(outs[g]).reshape(NPROB, W + 1).astype(np.float64)
        Bo = np.asarray(outs[g + 4]).reshape(NPROB, W + 1).astype(np.float64)
        Frow, lF = Fo[:, 0:W], Fo[:, W]
        Brow, lB = Bo[:, 0:W], Bo[:, W]
        rev = Brow[:, ::-1]
        shift = np.concatenate([np.zeros((NPROB, 1)), rev[:, :-1]], 1)
        S = (Frow * (rev + shift)).sum(1)
        R = -2.0 * (np.log(S) - lF - lB)
        loss[NBAT * g:NBAT * (g + 1)] = (
            R[0:NBAT] - 0.5 * (R[NBAT:2 * NBAT] + R[2 * NBAT:])
        ).astype(np.float32)
    return loss


def kernel(x: np.ndarray, y: np.ndarray) -> np.ndarray:
    x = np.ascontiguousarray(x, np.float32)
    y = np.ascontiguousarray(y, np.float32)
    B = x.shape[0]
    assert x.shape == (B, N, DIM) and B == NBAT * 4
    nc = _get_nc()
    res = run_bass_kernel_spmd(nc, _in_maps(x, y), list(range(NCORE)))
    outs = [res.results[k]["out"] for k in range(NCORE)]
    return _combine(outs)


if __name__ == "__main__":
    xx = np.random.randn(32, N, DIM).astype(np.float32)
    yy = np.random.randn(32, N, DIM).astype(np.float32)
    print(kernel(xx, yy)[:4])


# revision 18
# speedup vs baseline: 1.1571x; 1.1571x over previous
"""Banded soft-DTW loss (normalize=True) Trainium2 Bass kernel, v2.

Problem: x, y [32, 512, 4] f32 -> loss [32] f32
  loss = softdtw(x,y) - 0.5*(softdtw(x,x) + softdtw(y,y)), gamma=2, band=50.

Strategy v2 — forward/backward split (halves the serial DP chain):
  * The soft-DTW path-sum factors at the middle cut:
      S_total = sum_j F[255,j] * (B[256,j] + B[256,j+1])
    where F is the forward exp-space DP after rows 0..255 and B the
    backward DP, which equals the forward DP of the REVERSED sequences.
  * Cores 0-3 run forward halves, cores 4-7 run the same program on
    host-reversed inputs. Each core: 24 DP problems (8 batches x
    {xy,xx,yy}) x 256 rows, batched across 24 SBUF partitions.
  * Cost matrices: one K=6 augmented matmul per (problem, 128-row chunk)
    computes G' = -D/2 over a 228-wide j-window; ACT exp -> E = exp(-D/2)
    bf16; DRAM round-trip extracts the 101-wide diagonal band per row.
  * Exp-space DP row scan: S_i[t] = E_i[t]*(S_i[t-1] + S_{i-1}[t+1] +
    S_{i-1}[t]) via tensor_add + tensor_tensor_scan per row on DVE.
    Rescale by the diagonal cell every RS rows (log accumulated).
  * Each core outputs its boundary row + log-rescale sum; the host does
    the tiny O(B*W) combine.
"""
import os
import sys
from contextlib import ExitStack

import numpy as np

for _p in ("/opt/trn_rl_repo", "/root/.axon_site/_ro/trn_rl_repo"):
    if os.path.isdir(_p) and _p not in sys.path:
        sys.path.append(_p)

import concourse.bass as bass
import concourse.bacc as bacc
import concourse.mybir as mybir
import concourse.tile as tile
from concourse.bass_utils import run_bass_kernel_spmd

F32 = mybir.dt.float32
BF16 = mybir.dt.bfloat16
ALU = mybir.AluOpType
ACTF = mybir.ActivationFunctionType

N = 512            # full sequence length
HN = 256           # rows per half (per core)
DIM = 4
NBAT = 8           # batch elements per core
NSEQ = 16          # sequences per core (8 x + 8 y)
NPROB = 24         # DP problems per core (xy, xx, yy for 8 batches)
NCORE = 8
BW = 50            # band half width
W = 101            # band window width per row
SW = 104           # S tile width (101 + 3 zero guard cols)
NCHUNK = 2         # 128-row chunks per half
WIN = 228          # matmul j-window: 128 + 101 - 1
ACOLS = HN         # a-side cols per sequence
AV = HN + BW       # valid b prefix length (306)
BCOLS = 356        # b cols per sequence: 50 left pad + 306
RS = 8             # rescale cadence (rows)
GROWS = 32         # E-tile group rows
NGRP = HN // GROWS
NEVT = HN // RS    # 32 rescale events
CAP = 1e30
NEG = -20000.0     # pad dot-product value -> exp() == 0


def _groups():
    # (chunk, first_row, nrows): 8-row leading groups so the DP can start
    # as soon as the first few rows of E are gathered.
    gs = []
    for c in range(NCHUNK):
        sizes = [8, 8, 8, 8, 32, 32, 32] if c == 0 else [32, 32, 32, 32]
        r = c * 128
        for n in sizes:
            gs.append((c, r, n))
            r += n
    return gs


GRPS = _groups()
ROW2G = {}
for _gi, (_c, _r0, _n) in enumerate(GRPS):
    for _r in range(_r0, _r0 + _n):
        ROW2G[_r] = (_gi, _r - _r0)


def _build_nc():
    nc = bacc.Bacc("TRN2", target_bir_lowering=False, debug=False)
    xt = nc.dram_tensor("xt", [NBAT, DIM, N], F32, kind="ExternalInput").ap()
    yt = nc.dram_tensor("yt", [NBAT, DIM, N], F32, kind="ExternalInput").ap()
    nhc = nc.dram_tensor("nhc", [5, 12], F32, kind="ExternalInput").ap()
    out = nc.dram_tensor("out", [NPROB, W + 1], F32, kind="ExternalOutput").ap()

    with tile.TileContext(nc) as tc, ExitStack() as ctx:
        _emit(ctx, tc, xt, yt, nhc, out)
    nc.compile()
    return nc


def _emit(ctx, tc, xt, yt, nhc, out):
    nc = tc.nc

    const = ctx.enter_context(tc.tile_pool(name="const", bufs=1))
    winp = ctx.enter_context(tc.tile_pool(name="winp", bufs=26))
    ps_win = ctx.enter_context(tc.tile_pool(name="ps_win", bufs=6, space="PSUM"))
    ps_misc = ctx.enter_context(tc.tile_pool(name="ps_misc", bufs=1, space="PSUM"))
    dramp = ctx.enter_context(tc.tile_pool(name="dramp", bufs=1, space="DRAM"))
    epool = ctx.enter_context(tc.tile_pool(name="epool", bufs=1))

    scratch = dramp.tile([NPROB, NCHUNK, 128, WIN], BF16)
    e_tiles = [
        epool.tile([NPROB, n * W], BF16, name=f"edp{gi}", tag=f"edp{gi}")
        for gi, (c, r0, n) in enumerate(GRPS)
    ]

    QQ = [nc.sync, nc.scalar, nc.gpsimd]

    # DP state tiles are allocated BEFORE the prep pool so they do not
    # reuse its SBUF (which would add a WAR dependency on the whole prep).
    spool = ctx.enter_context(tc.tile_pool(name="spool", bufs=1))
    cbp = ctx.enter_context(tc.tile_pool(name="cbp", bufs=2))
    m_buf = const.tile([NPROB, NEVT], F32)
    s_ring = [
        spool.tile([NPROB, SW], BF16, tag=f"s{k}", name=f"s{k}") for k in range(3)
    ]
    for s in s_ring:
        nc.vector.memset(s[:], 0.0)

    # ---- Phase 0+1 fused: K=6 operand prep pipelined with chunk-0 ---------
    # a6 rows: [a0..a3, ones, -|a|^2/2]; b6 rows: [b0..b3, -|b|^2/2 (NEG
    # pads), ones].  G' = a6 . b6 = a.b - |a|^2/2 - |b|^2/2 = -D/2.
    # Norm rows are produced as 6-partition PSUM matmul outputs (rows 0-3
    # zero) and ACT-copied over a6/b6 BEFORE the feature casts land.
    with tc.tile_pool(name="pre", bufs=1) as pre:
        a6 = pre.tile([6, NSEQ * ACOLS], BF16)
        b6 = pre.tile([6, NSEQ * BCOLS], BF16)
        nh_f = pre.tile([5, 12], F32)
        nh_bf = pre.tile([5, 12], BF16)
        stag = pre.tile([4, NSEQ * AV], F32)
        sq = pre.tile([5, NSEQ * AV], BF16)
        ones_stg = pre.tile([1, NSEQ * AV], BF16)
        npad = pre.tile([1, NBAT * BW * 2], BF16)

        # nha col4 picks the ones row, col5 = -0.5*sum(sq); nhb swapped.
        # Pattern comes in as a tiny host input (engine ops cannot write
        # single partitions above 0).
        nc.gpsimd.dma_start(nh_f[:], nhc)
        nc.vector.tensor_copy(nh_bf[:], nh_f[:])
        nha = nh_bf[:, 0:6]
        nhb = nh_bf[:, 6:12]
        nc.vector.memset(ones_stg[:], 1.0)
        nc.vector.memset(npad[:], NEG)
        # sq row 4 = ones (via DMA: engine can't write partition 4 alone)
        nc.gpsimd.dma_start(sq[4:5, :], ones_stg[:])
        # b6 row-4 NEG pads and row-5 ones pads for all seqs in 2 DMAs
        b45 = b6[4:6, :].rearrange("p (s c) -> p s c", c=BCOLS)
        nc.gpsimd.dma_start(
            b45[0:1, :, 0:BW],
            npad[:].rearrange("p (s c) -> p s c", c=BW))
        nc.gpsimd.dma_start(
            b45[1:2, :, 0:BW],
            ones_stg[:, 0:NSEQ * BW].rearrange("p (s c) -> p s c", c=BW))

        st3 = stag[:].rearrange("p (s c) -> p s c", c=AV)
        sq3 = sq[0:4, :].rearrange("p (s c) -> p s c", c=AV)
        a3 = a6[0:4, :].rearrange("p (s c) -> p s c", c=ACOLS)
        b3 = b6[0:4, :].rearrange("p (s c) -> p s c", c=BCOLS)
        xsrc = xt.transpose([1, 0, 2])
        ysrc = yt.transpose([1, 0, 2])
        nc.sync.dma_start(st3[:, 0:NBAT, :], xsrc[:, :, 0:AV])
        nc.scalar.dma_start(st3[:, NBAT:NSEQ, :], ysrc[:, :, 0:AV])

        pairs = [(b, NBAT + b) for b in range(NBAT)] \
            + [(b, b) for b in range(NBAT)] \
            + [(NBAT + b, NBAT + b) for b in range(NBAT)]
        sc_handle = scratch[:].tensor

        ew_tiles = {}

        def emit_problem(pi, c):
            sa, sb = pairs[pi]
            pw = ps_win.tile([128, WIN], F32, name=f"pw{pi}_{c}", tag="pw")
            nc.tensor.matmul(
                pw[:],
                a6[:, sa * ACOLS + c * 128:sa * ACOLS + (c + 1) * 128],
                b6[:, sb * BCOLS + c * 128:sb * BCOLS + c * 128 + WIN],
                start=True, stop=True,
            )
            ew = winp.tile([128, WIN], BF16, name=f"ew{pi}_{c}", tag="ew")
            nc.scalar.activation(ew[:], pw[:], ACTF.Exp)
            ew_tiles[pi] = ew

        # per-seq prep pipeline: mul (DVE) -> 2 norm matmuls (PE) ->
        # 2 ACT copies -> feature casts (DVE); then this batch's chunk-0
        # problems immediately so the E pipeline starts while later
        # sequences are still being prepped.
        for b in range(NBAT):
            for s in (b, NBAT + b):
                nc.vector.tensor_mul(sq3[:, s, :], st3[:, s, :], st3[:, s, :])
                pna = ps_misc.tile([6, ACOLS], F32, name=f"pna{s}", tag="pna")
                pnb = ps_misc.tile([6, AV], F32, name=f"pnb{s}", tag="pnb")
                nc.tensor.matmul(
                    pna[:], nha[:], sq[:, s * AV:s * AV + ACOLS],
                    start=True, stop=True)
                nc.tensor.matmul(
                    pnb[:], nhb[:], sq[:, s * AV:(s + 1) * AV],
                    start=True, stop=True)
                nc.scalar.copy(a6[:, s * ACOLS:(s + 1) * ACOLS], pna[:])
                nc.scalar.copy(b6[:, s * BCOLS + BW:(s + 1) * BCOLS], pnb[:])
                nc.vector.tensor_copy(a3[:, s, :], st3[:, s, 0:ACOLS])
                nc.vector.tensor_copy(b3[:, s, BW:BCOLS], st3[:, s, :])
                nc.vector.memset(b3[:, s, 0:BW], 0.0)
            emit_problem(b, 0)            # xy
            emit_problem(NBAT + b, 0)     # xx
            emit_problem(2 * NBAT + b, 0) # yy

        def emit_slab_writes_gathers(c):
            # slab-major: each group's rows written for every problem, then
            # that group's gather — the DP's first group unblocks right
            # after the exps instead of after all full-window writes.
            for gi, (cc, r0, n) in enumerate(GRPS):
                if cc != c:
                    continue
                rl = r0 - c * 128
                for pi in range(NPROB):
                    QQ[pi % 3].dma_start(
                        scratch[pi, c, rl:rl + n],
                        ew_tiles[pi][rl:rl + n, :])
                e3g = e_tiles[gi][:].rearrange("p (r t) -> p r t", t=W)
                for q in range(3):
                    p0 = 8 * q
                    src = bass.AP(
                        sc_handle,
                        p0 * (NCHUNK * 128 * WIN) + c * 128 * WIN
                        + rl * (WIN + 1),
                        [[NCHUNK * 128 * WIN, 8], [WIN + 1, n], [1, W]],
                    )
                    QQ[q].dma_start(e3g[p0:p0 + 8], src)

        emit_slab_writes_gathers(0)
        for pi in range(NPROB):
            emit_problem(pi, 1)
        emit_slab_writes_gathers(1)

    # ---- Phase 3: exp-space row-scan DP (256 rows) ------------------------
    DPROWS = int(os.environ.get('KROWS', str(HN)))
    for i in range(DPROWS):
        gi, rl = ROW2G[i]
        e3g = e_tiles[gi][:].rearrange("p (r t) -> p r t", t=W)
        e_row = e3g[:, rl, :]
        ct = cbp.tile([NPROB, W], BF16, tag="c", name=f"c{i}")
        if i == 0:
            nc.vector.memset(ct[:], 0.0)
            nc.vector.memset(ct[:, BW:BW + 1], 1.0)
        else:
            sp = s_ring[(i - 1) % 3]
            nc.vector.tensor_add(ct[:], sp[:, 1:W + 1], sp[:, 0:W])
        st = s_ring[i % 3]
        # state = (c[t] + state) * E[t]  — the full soft-DTW row recurrence
        nc.vector.tensor_tensor_scan(
            st[:, 0:W], ct[:], e_row, 0.0, ALU.add, ALU.mult
        )
        if i % RS == RS - 1:
            ev = i // RS
            # m_buf stores 1/m; readout subtracts sum(ln(1/m))
            nc.vector.reciprocal(m_buf[:, ev:ev + 1], st[:, BW:BW + 1])
            nc.vector.tensor_scalar(
                st[:, 0:W], st[:, 0:W], m_buf[:, ev:ev + 1], CAP,
                ALU.mult, ALU.min
            )

    # ---- Phase 4: readout — boundary row + log-rescale sum ----------------
    ln_m = const.tile([NPROB, NEVT], F32)
    obuf = const.tile([NPROB, W + 1], F32)
    nc.scalar.activation(ln_m[:], m_buf[:], ACTF.Ln)
    nc.vector.reduce_sum(obuf[:, W:W + 1], ln_m[:], axis=mybir.AxisListType.X)
    s_last = s_ring[(DPROWS - 1) % 3 if DPROWS else 0]
    nc.scalar.copy(obuf[:, 0:W], s_last[:, 0:W])
    nc.sync.dma_start(out, obuf[:])


_NC_CACHE = None


def _get_nc():
    global _NC_CACHE
    if _NC_CACHE is None:
        _NC_CACHE = _build_nc()
    return _NC_CACHE


def _nhc_np():
    # [5, 12] = nha | nhb, contraction rows [sq0..sq3, ones]
    v = np.zeros((5, 12), np.float32)
    v[0:4, 5] = -0.5   # nha col5: -|a|^2/2
    v[4, 4] = 1.0      # nha col4: ones row
    v[0:4, 6 + 4] = -0.5  # nhb col4: -|b|^2/2
    v[4, 6 + 5] = 1.0     # nhb col5: ones row
    return v


def _in_maps(x, y):
    """Per-core inputs: cores 0-3 forward batches 8g..8g+7, cores 4-7 the
    same batches with sequences reversed (backward half)."""
    maps = []
    for c in range(NCORE):
        g = c % 4
        xs = x[NBAT * g:NBAT * (g + 1)].transpose(0, 2, 1)
        ys = y[NBAT * g:NBAT * (g + 1)].transpose(0, 2, 1)
        if c >= 4:
            xs = xs[:, :, ::-1]
            ys = ys[:, :, ::-1]
        maps.append({
            "xt": np.ascontiguousarray(xs),
            "yt": np.ascontiguousarray(ys),
            "nhc": _nhc_np(),
        })
    return maps


def _combine(outs):
    """Host combine: S_total = sum_t F[t]*(Bp[101-t] + Bp[100-t]);
    loss = R_xy - (R_xx + R_yy)/2 with R = -2*(ln S - tsumF - tsumB)."""
    loss = np.zeros(NBAT * 4, np.float32)
    for g in range(4):
        Fo = np.asarray(outs[g]).reshape(NPROB, W + 1).astype(np.float64)
        Bo = np.asarray(outs[g + 4]).reshape(NPROB, W + 1).astype(np.float64)
        Frow, lF = Fo[:, 0:W], Fo[:, W]
        Brow, lB = Bo[:, 0:W], Bo[:, W]
        rev = Brow[:, ::-1]
        shift = np.concatenate([np.zeros((NPROB, 1)), rev[:, :-1]], 1)
        S = (Frow * (rev + shift)).sum(1)
        R = -2.0 * (np.log(S) - lF - lB)
        loss[NBAT * g:NBAT * (g + 1)] = (
            R[0:NBAT] - 0.5 * (R[NBAT:2 * NBAT] + R[2 * NBAT:])
        ).astype(np.float32)
    return loss


def kernel(x: np.ndarray, y: np.ndarray) -> np.ndarray:
    x = np.ascontiguousarray(x, np.float32)
    y = np.ascontiguousarray(y, np.float32)
    B = x.shape[0]
    assert x.shape == (B, N, DIM) and B == NBAT * 4
    nc = _get_nc()
    res = run_bass_kernel_spmd(nc, _in_maps(x, y), list(range(NCORE)))
    outs = [res.results[k]["out"] for k in range(NCORE)]
    return _combine(outs)


if __name__ == "__main__":
    xx = np.random.randn(32, N, DIM).astype(np.float32)
    yy = np.random.randn(32, N, DIM).astype(np.float32)
    print(kernel(xx, yy)[:4])


# revision 19
# speedup vs baseline: 1.1994x; 1.0366x over previous
"""Banded soft-DTW loss (normalize=True) Trainium2 Bass kernel, v2.

Problem: x, y [32, 512, 4] f32 -> loss [32] f32
  loss = softdtw(x,y) - 0.5*(softdtw(x,x) + softdtw(y,y)), gamma=2, band=50.

Strategy v2 — forward/backward split (halves the serial DP chain):
  * The soft-DTW path-sum factors at the middle cut:
      S_total = sum_j F[255,j] * (B[256,j] + B[256,j+1])
    where F is the forward exp-space DP after rows 0..255 and B the
    backward DP, which equals the forward DP of the REVERSED sequences.
  * Cores 0-3 run forward halves, cores 4-7 run the same program on
    host-reversed inputs. Each core: 24 DP problems (8 batches x
    {xy,xx,yy}) x 256 rows, batched across 24 SBUF partitions.
  * Cost matrices: one K=6 augmented matmul per (problem, 128-row chunk)
    computes G' = -D/2 over a 228-wide j-window; ACT exp -> E = exp(-D/2)
    bf16; DRAM round-trip extracts the 101-wide diagonal band per row.
  * Exp-space DP row scan: S_i[t] = E_i[t]*(S_i[t-1] + S_{i-1}[t+1] +
    S_{i-1}[t]) via tensor_add + tensor_tensor_scan per row on DVE.
    Rescale by the diagonal cell every RS rows (log accumulated).
  * Each core outputs its boundary row + log-rescale sum; the host does
    the tiny O(B*W) combine.
"""
import os
import sys
from contextlib import ExitStack

import numpy as np

for _p in ("/opt/trn_rl_repo", "/root/.axon_site/_ro/trn_rl_repo"):
    if os.path.isdir(_p) and _p not in sys.path:
        sys.path.append(_p)

import concourse.bass as bass
import concourse.bacc as bacc
import concourse.mybir as mybir
import concourse.tile as tile
from concourse.bass_utils import run_bass_kernel_spmd

F32 = mybir.dt.float32
BF16 = mybir.dt.bfloat16
ALU = mybir.AluOpType
ACTF = mybir.ActivationFunctionType

N = 512            # full sequence length
HN = 256           # rows per half (per core)
DIM = 4
NBAT = 8           # batch elements per core
NSEQ = 16          # sequences per core (8 x + 8 y)
NPROB = 24         # DP problems per core (xy, xx, yy for 8 batches)
NCORE = 8
BW = 50            # band half width
W = 101            # band window width per row
SW = 104           # S tile width (101 + 3 zero guard cols)
NCHUNK = 2         # 128-row chunks per half
WIN = 228          # matmul j-window: 128 + 101 - 1
ACOLS = HN         # a-side cols per sequence
AV = HN + BW       # valid b prefix length (306)
BCOLS = 356        # b cols per sequence: 50 left pad + 306
RS = 8             # rescale cadence (rows)
GROWS = 32         # E-tile group rows
NGRP = HN // GROWS
NEVT = HN // RS    # 32 rescale events
CAP = 1e30
NEG = -20000.0     # pad dot-product value -> exp() == 0


def _build_nc():
    nc = bacc.Bacc("TRN2", target_bir_lowering=False, debug=False)
    xt = nc.dram_tensor("xt", [NBAT, DIM, N], F32, kind="ExternalInput").ap()
    yt = nc.dram_tensor("yt", [NBAT, DIM, N], F32, kind="ExternalInput").ap()
    nhc = nc.dram_tensor("nhc", [5, 12], F32, kind="ExternalInput").ap()
    out = nc.dram_tensor("out", [NPROB, W + 1], F32, kind="ExternalOutput").ap()

    with tile.TileContext(nc) as tc, ExitStack() as ctx:
        _emit(ctx, tc, xt, yt, nhc, out)
    nc.compile()
    return nc


def _emit(ctx, tc, xt, yt, nhc, out):
    nc = tc.nc

    const = ctx.enter_context(tc.tile_pool(name="const", bufs=1))
    winp = ctx.enter_context(tc.tile_pool(name="winp", bufs=14))
    ps_win = ctx.enter_context(tc.tile_pool(name="ps_win", bufs=6, space="PSUM"))
    ps_misc = ctx.enter_context(tc.tile_pool(name="ps_misc", bufs=1, space="PSUM"))
    dramp = ctx.enter_context(tc.tile_pool(name="dramp", bufs=1, space="DRAM"))
    epool = ctx.enter_context(tc.tile_pool(name="epool", bufs=1))

    scratch = dramp.tile([NPROB, NCHUNK, 128, WIN], BF16)
    e_tiles = [
        epool.tile([NPROB, GROWS * W], BF16, name=f"edp{g}", tag=f"edp{g}")
        for g in range(NGRP)
    ]

    QQ = [nc.sync, nc.scalar, nc.gpsimd]

    # DP state tiles are allocated BEFORE the prep pool so they do not
    # reuse its SBUF (which would add a WAR dependency on the whole prep).
    spool = ctx.enter_context(tc.tile_pool(name="spool", bufs=1))
    cbp = ctx.enter_context(tc.tile_pool(name="cbp", bufs=2))
    m_buf = const.tile([NPROB, NEVT], F32)
    s_ring = [
        spool.tile([NPROB, SW], BF16, tag=f"s{k}", name=f"s{k}") for k in range(3)
    ]
    for s in s_ring:
        nc.vector.memset(s[:], 0.0)

    # ---- Phase 0+1 fused: K=6 operand prep pipelined with chunk-0 ---------
    # a6 rows: [a0..a3, ones, -|a|^2/2]; b6 rows: [b0..b3, -|b|^2/2 (NEG
    # pads), ones].  G' = a6 . b6 = a.b - |a|^2/2 - |b|^2/2 = -D/2.
    # Norm rows are produced as 6-partition PSUM matmul outputs (rows 0-3
    # zero) and ACT-copied over a6/b6 BEFORE the feature casts land.
    with tc.tile_pool(name="pre", bufs=1) as pre:
        a6 = pre.tile([6, NSEQ * ACOLS], BF16)
        b6 = pre.tile([6, NSEQ * BCOLS], BF16)
        nh_f = pre.tile([5, 12], F32)
        nh_bf = pre.tile([5, 12], BF16)
        stag = pre.tile([4, NSEQ * AV], F32)
        sq = pre.tile([5, NSEQ * AV], BF16)
        ones_stg = pre.tile([1, NSEQ * AV], BF16)
        npad = pre.tile([1, NBAT * BW * 2], BF16)

        # nha col4 picks the ones row, col5 = -0.5*sum(sq); nhb swapped.
        # Pattern comes in as a tiny host input (engine ops cannot write
        # single partitions above 0).
        nc.gpsimd.dma_start(nh_f[:], nhc)
        nc.vector.tensor_copy(nh_bf[:], nh_f[:])
        nha = nh_bf[:, 0:6]
        nhb = nh_bf[:, 6:12]
        nc.vector.memset(ones_stg[:], 1.0)
        nc.vector.memset(npad[:], NEG)
        # sq row 4 = ones (via DMA: engine can't write partition 4 alone)
        nc.gpsimd.dma_start(sq[4:5, :], ones_stg[:])
        # b6 row-4 NEG pads and row-5 ones pads for all seqs in 2 DMAs
        b45 = b6[4:6, :].rearrange("p (s c) -> p s c", c=BCOLS)
        nc.gpsimd.dma_start(
            b45[0:1, :, 0:BW],
            npad[:].rearrange("p (s c) -> p s c", c=BW))
        nc.gpsimd.dma_start(
            b45[1:2, :, 0:BW],
            ones_stg[:, 0:NSEQ * BW].rearrange("p (s c) -> p s c", c=BW))

        st3 = stag[:].rearrange("p (s c) -> p s c", c=AV)
        sq3 = sq[0:4, :].rearrange("p (s c) -> p s c", c=AV)
        a3 = a6[0:4, :].rearrange("p (s c) -> p s c", c=ACOLS)
        b3 = b6[0:4, :].rearrange("p (s c) -> p s c", c=BCOLS)
        xsrc = xt.transpose([1, 0, 2])
        ysrc = yt.transpose([1, 0, 2])
        nc.sync.dma_start(st3[:, 0:NBAT, :], xsrc[:, :, 0:AV])
        nc.scalar.dma_start(st3[:, NBAT:NSEQ, :], ysrc[:, :, 0:AV])

        pairs = [(b, NBAT + b) for b in range(NBAT)] \
            + [(b, b) for b in range(NBAT)] \
            + [(NBAT + b, NBAT + b) for b in range(NBAT)]
        sc_handle = scratch[:].tensor

        ew_tiles = {}
        pend = []

        def emit_problem(pi, c):
            # two problems share a PSUM tile; one ACT exp covers both
            pend.append((pi, c))
            if len(pend) < 2:
                return
            (p0i, c0), (p1i, c1) = pend
            pend.clear()
            pw = ps_win.tile([128, 2 * WIN], F32,
                             name=f"pw{p0i}_{c0}", tag="pw")
            ew = winp.tile([128, 2 * WIN], BF16,
                           name=f"ew{p0i}_{c0}", tag="ew")
            for k, (pj, cj) in enumerate(((p0i, c0), (p1i, c1))):
                sa, sb = pairs[pj]
                nc.tensor.matmul(
                    pw[:, k * WIN:(k + 1) * WIN],
                    a6[:, sa * ACOLS + cj * 128:sa * ACOLS + (cj + 1) * 128],
                    b6[:, sb * BCOLS + cj * 128:sb * BCOLS + cj * 128 + WIN],
                    start=True, stop=True,
                )
                ew_tiles[(pj, cj)] = ew[:, k * WIN:(k + 1) * WIN]
            nc.scalar.activation(ew[:], pw[:], ACTF.Exp)

        # per-seq prep pipeline: mul (DVE) -> 2 norm matmuls (PE) ->
        # 2 ACT copies -> feature casts (DVE); then this batch's chunk-0
        # problems immediately so the E pipeline starts while later
        # sequences are still being prepped.
        for b in range(NBAT):
            for s in (b, NBAT + b):
                nc.vector.tensor_mul(sq3[:, s, :], st3[:, s, :], st3[:, s, :])
                pna = ps_misc.tile([6, ACOLS], F32, name=f"pna{s}", tag="pna")
                pnb = ps_misc.tile([6, AV], F32, name=f"pnb{s}", tag="pnb")
                nc.tensor.matmul(
                    pna[:], nha[:], sq[:, s * AV:s * AV + ACOLS],
                    start=True, stop=True)
                nc.tensor.matmul(
                    pnb[:], nhb[:], sq[:, s * AV:(s + 1) * AV],
                    start=True, stop=True)
                nc.vector.tensor_copy(a6[:, s * ACOLS:(s + 1) * ACOLS], pna[:])
                nc.scalar.copy(b6[:, s * BCOLS + BW:(s + 1) * BCOLS], pnb[:])
                nc.vector.tensor_copy(a3[:, s, :], st3[:, s, 0:ACOLS])
                nc.vector.tensor_copy(b3[:, s, BW:BCOLS], st3[:, s, :])
                nc.vector.memset(b3[:, s, 0:BW], 0.0)
            emit_problem(b, 0)            # xy
            emit_problem(NBAT + b, 0)     # xx
            emit_problem(2 * NBAT + b, 0) # yy

        def emit_slab_writes_gathers(c):
            # slab-major: rows [32h, 32h+32) of every problem written first,
            # then that group's gather — the DP's first group unblocks right
            # after the exps instead of after all full-window writes.
            for h in range(128 // GROWS):
                g = c * (128 // GROWS) + h
                for pi in range(NPROB):
                    QQ[pi % 3].dma_start(
                        scratch[pi, c, h * GROWS:(h + 1) * GROWS],
                        ew_tiles[(pi, c)][h * GROWS:(h + 1) * GROWS, :])
                e3g = e_tiles[g][:].rearrange("p (r t) -> p r t", t=W)
                for q in range(3):
                    p0 = 8 * q
                    src = bass.AP(
                        sc_handle,
                        p0 * (NCHUNK * 128 * WIN) + c * 128 * WIN
                        + h * GROWS * (WIN + 1),
                        [[NCHUNK * 128 * WIN, 8], [WIN + 1, GROWS], [1, W]],
                    )
                    QQ[q].dma_start(e3g[p0:p0 + 8], src)

        emit_slab_writes_gathers(0)
        for pi in range(NPROB):
            emit_problem(pi, 1)
        emit_slab_writes_gathers(1)

    # ---- Phase 3: exp-space row-scan DP (256 rows) ------------------------
    DPROWS = int(os.environ.get('KROWS', str(HN)))
    for i in range(DPROWS):
        e3g = e_tiles[i // GROWS][:].rearrange("p (r t) -> p r t", t=W)
        e_row = e3g[:, i % GROWS, :]
        ct = cbp.tile([NPROB, W], BF16, tag="c", name=f"c{i}")
        if i == 0:
            nc.vector.memset(ct[:], 0.0)
            nc.vector.memset(ct[:, BW:BW + 1], 1.0)
        else:
            sp = s_ring[(i - 1) % 3]
            nc.vector.tensor_add(ct[:], sp[:, 1:W + 1], sp[:, 0:W])
        st = s_ring[i % 3]
        # state = (c[t] + state) * E[t]  — the full soft-DTW row recurrence
        nc.vector.tensor_tensor_scan(
            st[:, 0:W], ct[:], e_row, 0.0, ALU.add, ALU.mult
        )
        if i % RS == RS - 1:
            ev = i // RS
            # m_buf stores 1/m; readout subtracts sum(ln(1/m))
            nc.vector.reciprocal(m_buf[:, ev:ev + 1], st[:, BW:BW + 1])
            nc.vector.tensor_scalar(
                st[:, 0:W], st[:, 0:W], m_buf[:, ev:ev + 1], CAP,
                ALU.mult, ALU.min
            )

    # ---- Phase 4: readout — boundary row + log-rescale sum ----------------
    ln_m = const.tile([NPROB, NEVT], F32)
    obuf = const.tile([NPROB, W + 1], F32)
    nc.scalar.activation(ln_m[:], m_buf[:], ACTF.Ln)
    nc.vector.reduce_sum(obuf[:, W:W + 1], ln_m[:], axis=mybir.AxisListType.X)
    s_last = s_ring[(DPROWS - 1) % 3 if DPROWS else 0]
    nc.scalar.copy(obuf[:, 0:W], s_last[:, 0:W])
    nc.sync.dma_start(out, obuf[:])


_NC_CACHE = None


def _get_nc():
    global _NC_CACHE
    if _NC_CACHE is None:
        _NC_CACHE = _build_nc()
    return _NC_CACHE


def _nhc_np():
    # [5, 12] = nha | nhb, contraction rows [sq0..sq3, ones]
    v = np.zeros((5, 12), np.float32)
    v[0:4, 5] = -0.5   # nha col5: -|a|^2/2
    v[4, 4] = 1.0      # nha col4: ones row
    v[0:4, 6 + 4] = -0.5  # nhb col4: -|b|^2/2
    v[4, 6 + 5] = 1.0     # nhb col5: ones row
    return v


def _in_maps(x, y):
    """Per-core inputs: cores 0-3 forward batches 8g..8g+7, cores 4-7 the
    same batches with sequences reversed (backward half)."""
    maps = []
    for c in range(NCORE):
        g = c % 4
        xs = x[NBAT * g:NBAT * (g + 1)].transpose(0, 2, 1)
        ys = y[NBAT * g:NBAT * (g + 1)].transpose(0, 2, 1)
        if c >= 4:
            xs = xs[:, :, ::-1]
            ys = ys[:, :, ::-1]
        maps.append({
            "xt": np.ascontiguousarray(xs),
            "yt": np.ascontiguousarray(ys),
            "nhc": _nhc_np(),
        })
    return maps


def _combine(outs):
    """Host combine: S_total = sum_t F[t]*(Bp[101-t] + Bp[100-t]);
    loss = R_xy - (R_xx + R_yy)/2 with R = -2*(ln S - tsumF - tsumB)."""
    loss = np.zeros(NBAT * 4, np.float32)
    for g in range(4):
        Fo = np.asarray(outs[g]).reshape(NPROB, W + 1).astype(np.float64)
        Bo = np.asarray(outs[g + 4]).reshape(NPROB, W + 1).astype(np.float64)
        Frow, lF = Fo[:, 0:W], Fo[:, W]
        Brow, lB = Bo[:, 0:W], Bo[:, W]
        rev = Brow[:, ::-1]
        shift = np.concatenate([np.zeros((NPROB, 1)), rev[:, :-1]], 1)
        S = (Frow * (rev + shift)).sum(1)
        R = -2.0 * (np.log(S) - lF - lB)
        loss[NBAT * g:NBAT * (g + 1)] = (
            R[0:NBAT] - 0.5 * (R[NBAT:2 * NBAT] + R[2 * NBAT:])
        ).astype(np.float32)
    return loss


def kernel(x: np.ndarray, y: np.ndarray) -> np.ndarray:
    x = np.ascontiguousarray(x, np.float32)
    y = np.ascontiguousarray(y, np.float32)
    B = x.shape[0]
    assert x.shape == (B, N, DIM) and B == NBAT * 4
    nc = _get_nc()
    res = run_bass_kernel_spmd(nc, _in_maps(x, y), list(range(NCORE)))
    outs = [res.results[k]["out"] for k in range(NCORE)]
    return _combine(outs)


if __name__ == "__main__":
    xx = np.random.randn(32, N, DIM).astype(np.float32)
    yy = np.random.randn(32, N, DIM).astype(np.float32)
    print(kernel(xx, yy)[:4])
